# revision 11
# baseline (speedup 1.0000x reference)
"""Two-layer RGAT (R=3, heads=1) on 8 trn2 NeuronCores.

Strategy (dst-sharded, one-hot-matmul aggregation):
  - Nodes padded to 50176 = 8 cores x 49 blocks x 128; core c owns dst nodes
    [c*6272, (c+1)*6272) and computes the full output rows for them.
  - Per layer, each core computes its slice of the per-relation node transform
    xw[r] = x @ W_r (plus attention scalars ak = xw@k, aq = xw@q) into a DRAM
    table (row = (src_core, rt, src_local), 192-f32 stride, 130 payload:
    [128 feats | 1.0 | ak]); AllGather replicates the table.
  - Edges (sorted by dst block, then by table-row range so int16 gather
    indices fit) are processed in 128-edge chunks: dma_gather fetches the
    chunk's source rows; alpha = exp(LeakyRelu(aq[rt,dst] + ak[rt,src] +
    c_l*ea)) is built from a second (local) aq-table gather; a fused DVE
    tensor_scalar builds the alpha-scaled one-hot O[e, dst_local]; one
    matmul per chunk accumulates psum[node,129] = [sum alpha*xj | sum alpha].
  - Block results accumulate in SBUF across range-phases; finalize divides by
    the denominator, adds bias (+ReLU for layer 1). Layer-2 output rows DMA
    straight to the per-core output; the host concatenates and trims.

I/O path (the axon tunnel moves ~60-90 MB/s, so host<->device bytes dominate):
  - Inputs are sent compact: xT/W packs in fp16, dst-locals as int16, raw
    edge_attr as fp16 (scaled by c1/c2 on device), gather-index tiles as a
    single 16-partition copy (the 8x SWDGE replication is done on-device).
  - The output is fp16.
  - The jitted executable is cached; the NEFF's zero output buffers live on
    device persistently (no donation - the kernel writes every output elem).
"""
import sys
sys.path.insert(0, '/opt/trn_rl_repo')
import inspect
import textwrap
import numpy as np

import concourse.bass as bass
import concourse.bacc as bacc
import concourse.mybir as mybir
from concourse import bass2jax
from concourse.tile import TileContext
from concourse.masks import make_identity

F32 = mybir.dt.float32
F16 = mybir.dt.float16
I16 = mybir.dt.int16
I32 = mybir.dt.int32
NEG_SLOPE = 0.2

# ---- relax dma_gather's elem_size%256 restriction (descriptor length is ----
# ---- arbitrary; only the row *stride* must be a multiple of 256B)       ----
_src = inspect.getsource(bass.BassGpSimd.dma_gather)
_src = _src.replace(
    "elem_size_bytes > 0 and elem_size_bytes % 256 == 0",
    "elem_size_bytes > 0",
)
_ns = {}
exec(compile(textwrap.dedent(_src), "<dma_gather_patched>", "exec"), dict(vars(bass)), _ns)
bass.BassGpSimd.dma_gather = _ns["dma_gather"]


class Cfg:
    pass


def make_cfg(N, E, NC=8, GCALL=32, RANGE=32768):
    cfg = Cfg()
    cfg.NC = NC
    cfg.N, cfg.E = N, E
    cfg.NPAD = -(-N // (128 * NC)) * 128 * NC
    cfg.NPC = cfg.NPAD // NC
    cfg.NBLK = cfg.NPC // 128
    cfg.RPC = 3 * cfg.NPC
    cfg.RTOT = cfg.RPC * NC
    cfg.RANGE = RANGE
    cfg.NPH = -(-cfg.RTOT // RANGE)
    cfg.GCALL = GCALL
    return cfg


def host_prep(cfg, x, edge_index, edge_type, edge_attr, w1, q1, k1, le1, e1, b1,
              w2, q2, k2, le2, e2, b2):
    """Returns (per_core_inputs list, cfg with CPB/calls/NCH set)."""
    NC, NPC, NBLK, RANGE = cfg.NC, cfg.NPC, cfg.NBLK, cfg.RANGE
    src, dst = edge_index[0].astype(np.int64), edge_index[1].astype(np.int64)
    rt = edge_type.astype(np.int64)
    ea = edge_attr[:, 0].astype(np.float32)
    c1 = float(le1.reshape(-1) @ e1.reshape(-1))
    c2 = float(le2.reshape(-1) @ e2.reshape(-1))

    core = dst // NPC
    blk = (dst % NPC) // 128
    dl = dst % 128
    grow = (src // NPC) * cfg.RPC + rt * NPC + (src % NPC)
    ph = grow // RANGE
    lidx = grow - ph * RANGE
    aqi = rt * NPC + (dst % NPC)

    # per (core, blk, phase) counts -> CPB[p][b] = max-over-cores chunks
    counts = np.zeros((NC, NBLK, cfg.NPH), np.int64)
    np.add.at(counts, (core, blk, ph), 1)
    CPB = -(-counts.max(axis=0) // 128)          # [NBLK, NPH]
    cfg.CPB = CPB
    # slot layout: phase-major; within phase, blocks at cumsum offsets
    cfg.pboff = np.zeros((cfg.NPH, NBLK), np.int64)
    base = [0]
    for p in range(cfg.NPH):
        cfg.pboff[p] = np.concatenate([[0], np.cumsum(CPB[:-1, p])])
        base.append(base[-1] + int(CPB[:, p].sum()))
    cfg.base = np.asarray(base, np.int64)
    cfg.NCH = int(cfg.base[-1])

    # gather call list: per phase, contiguous slot runs of <= GCALL slots
    calls = []
    for p in range(cfg.NPH):
        nslots = int(CPB[:, p].sum())
        s = 0
        while s < nslots:
            ns = min(cfg.GCALL, nslots - s)
            calls.append((p, int(cfg.base[p] + s), int(ns)))
            s += ns
    cfg.calls = calls

    def pack16(vals):
        """vals [NCH*128] -> compact idx tile [16, NCH*8] (one SWDGE copy;
        the device replicates to the 8 required partition groups)."""
        out = np.zeros((16, cfg.NCH * 8), np.int16)
        for (p, s0, ns) in calls:
            v = vals[s0 * 128:(s0 + ns) * 128]
            i = np.arange(ns * 128)
            out[i % 16, s0 * 8 + i // 16] = v
        return out

    # weight packs (fp16)
    def wpack(w, qv, kv):
        W = np.zeros((128, 393), np.float32)
        for r in range(3):
            W[:, r * 130:r * 130 + 128] = w[r]
            W[:, r * 130 + 129] = (w[r] @ kv).ravel()
            W[:, 390 + r] = (w[r] @ qv).ravel()
        return W.astype(np.float16)

    W1p, W2p = wpack(w1, q1, k1), wpack(w2, q2, k2)

    # Concat-shaped base arrays; per-core dicts hold row-slice views so the
    # runner can skip the per-run np.concatenate memcpy.
    full = {
        "xT": np.zeros((NC * 128, NPC), np.float16),
        "W1": np.zeros((NC * 128, 393), np.float16),
        "W2": np.zeros((NC * 128, 393), np.float16),
        "B1": np.zeros((NC * 1, 128), np.float32),
        "B2": np.zeros((NC * 1, 128), np.float32),
        "CSC": np.zeros((NC * 1, 2), np.float32),
        "DSTS": np.full((NC * 128, cfg.NCH), -1, np.int16),
        "EAS": np.zeros((NC * 128, cfg.NCH), np.float16),
        "FIDX": np.zeros((NC * 16, cfg.NCH * 8), np.int16),
        "AQIX": np.zeros((NC * 16, cfg.NCH * 8), np.int16),
    }

    per_core = []
    for c in range(NC):
        m = core == c
        eb, ep = blk[m], ph[m]
        edl, elx, eaq = dl[m], lidx[m], aqi[m]
        eea = ea[m]
        order = np.lexsort((ep, eb))
        eb, ep, edl, elx, eaq, eea = (a[order] for a in (eb, ep, edl, elx, eaq, eea))
        # rank within (blk, phase) group
        gid = eb * cfg.NPH + ep
        boundaries = np.concatenate([[0], np.cumsum(np.bincount(gid.astype(np.int64),
                                                                minlength=NBLK * cfg.NPH))])
        rank = np.arange(len(gid)) - boundaries[gid]
        slot = cfg.base[ep] + cfg.pboff[ep, eb] + rank // 128
        prow = rank % 128

        dst_s = full["DSTS"][c * 128:(c + 1) * 128]
        ea_s = full["EAS"][c * 128:(c + 1) * 128]
        fidx_v = np.zeros(cfg.NCH * 128, np.int64)
        aq_v = np.zeros(cfg.NCH * 128, np.int64)
        dst_s[prow, slot] = edl
        ea_s[prow, slot] = eea
        fidx_v[slot * 128 + prow] = elx
        aq_v[slot * 128 + prow] = eaq
        full["FIDX"][c * 16:(c + 1) * 16] = pack16(fidx_v)
        full["AQIX"][c * 16:(c + 1) * 16] = pack16(aq_v)

        lo, hi = c * NPC, min((c + 1) * NPC, cfg.N)
        if hi > lo:
            full["xT"][c * 128:(c + 1) * 128, :hi - lo] = x[lo:hi].T.astype(np.float16)
        full["W1"][c * 128:(c + 1) * 128] = W1p
        full["W2"][c * 128:(c + 1) * 128] = W2p
        full["B1"][c] = b1.reshape(128).astype(np.float32)
        full["B2"][c] = b2.reshape(128).astype(np.float32)
        full["CSC"][c] = (c1, c2)
        per_core.append({k: (full[k][c * 16:(c + 1) * 16] if k in ("FIDX", "AQIX")
                             else full[k][c:c + 1] if k in ("B1", "B2", "CSC")
                             else full[k][c * 128:(c + 1) * 128])
                         for k in full})
    return per_core


def build_nc(cfg, skips=()):
    skips = set(skips)
    nc = bacc.Bacc("TRN2", target_bir_lowering=False, num_swdge_queues=4)
    NPC, NBLK, NCH = cfg.NPC, cfg.NBLK, cfg.NCH

    xT = nc.declare_dram_parameter("xT", [128, NPC], F16, isOutput=False)
    W = {1: nc.declare_dram_parameter("W1", [128, 393], F16, isOutput=False),
         2: nc.declare_dram_parameter("W2", [128, 393], F16, isOutput=False)}
    B = {1: nc.declare_dram_parameter("B1", [1, 128], F32, isOutput=False),
         2: nc.declare_dram_parameter("B2", [1, 128], F32, isOutput=False)}
    CSC = nc.declare_dram_parameter("CSC", [1, 2], F32, isOutput=False)
    DSTS = nc.declare_dram_parameter("DSTS", [128, NCH], I16, isOutput=False)
    EAS = nc.declare_dram_parameter("EAS", [128, NCH], F16, isOutput=False)
    FIDX = nc.declare_dram_parameter("FIDX", [16, NCH * 8], I16, isOutput=False)
    AQIX = nc.declare_dram_parameter("AQIX", [16, NCH * 8], I16, isOutput=False)
    OUT2 = nc.declare_dram_parameter("out2", [NPC, 128], F16, isOutput=True)

    BF16 = mybir.dt.bfloat16
    tabs = {L: nc.dram_tensor(f"tabs{L}", [cfg.RPC, 256], BF16) for L in (1, 2)}
    tabg = {L: nc.dram_tensor(f"tabg{L}", [cfg.RTOT, 256], BF16, addr_space="Shared")
            for L in (1, 2)}
    aqt = {L: nc.dram_tensor(f"aqt{L}", [cfg.RPC, 64], F32) for L in (1, 2)}

    AL = mybir.AluOpType
    AF = mybir.ActivationFunctionType

    with TileContext(nc) as tc:
        with (
            tc.tile_pool(name="const", bufs=1) as cp,
            tc.tile_pool(name="stag", bufs=4) as sp,
            tc.tile_pool(name="aqs", bufs=6) as qp,
            tc.tile_pool(name="oa", bufs=8) as op,
            tc.tile_pool(name="work", bufs=3) as wp,
            tc.tile_pool(name="pacc", bufs=4, space="PSUM") as pa,
            tc.tile_pool(name="ptab", bufs=2, space="PSUM") as pt,
            tc.tile_pool(name="pmisc", bufs=2, space="PSUM") as px,
        ):
            # ---- constants / staged inputs ----
            xT_t = cp.tile([128, NPC], F16)
            nc.sync.dma_start(out=xT_t[:], in_=xT[:])
            W_t = {L: cp.tile([128, 393], F16, tag=f"W{L}", name=f"W{L}_t") for L in (1, 2)}
            B_t = {L: cp.tile([1, 128], F32, tag=f"B{L}", name=f"B{L}_t") for L in (1, 2)}
            for L in (1, 2):
                nc.sync.dma_start(out=W_t[L][:], in_=W[L][:])
                nc.sync.dma_start(out=B_t[L][:], in_=B[L][:])
            csc_t = cp.tile([1, 2], F32)
            nc.sync.dma_start(out=csc_t[:], in_=CSC[:])
            dsti_t = cp.tile([128, NCH], I16)
            nc.sync.dma_start(out=dsti_t[:], in_=DSTS[:])
            ea_t = cp.tile([128, NCH], F16)
            nc.sync.dma_start(out=ea_t[:], in_=EAS[:])
            fidx_t = cp.tile([128, NCH * 8], I16)
            aqix_t = cp.tile([128, NCH * 8], I16)
            for g in range(8):
                nc.sync.dma_start(out=fidx_t[16 * g:16 * (g + 1), :], in_=FIDX[:])
                nc.sync.dma_start(out=aqix_t[16 * g:16 * (g + 1), :], in_=AQIX[:])

            ii = cp.tile([128, 128], I32)
            nc.gpsimd.iota(ii[:], pattern=[[1, 128]], base=0, channel_multiplier=0)
            iof = cp.tile([128, 128], F32)
            nc.vector.tensor_copy(iof[:], ii[:])
            ident = cp.tile([128, 128], F32)
            make_identity(nc, ident[:])
            ones1 = cp.tile([1, 128], F32)
            nc.vector.memset(ones1[:], 1.0)

            # dst-locals as f32; per-layer scaled edge attrs
            dst_t = cp.tile([128, NCH], F32)
            nc.vector.tensor_copy(dst_t[:], dsti_t[:])
            csb_p = px.tile([128, 2], F32, tag="pmisc")
            nc.tensor.matmul(csb_p[:], lhsT=ones1[:], rhs=csc_t[:], start=True, stop=True)
            csb = cp.tile([128, 2], F32)
            nc.vector.tensor_copy(csb[:], csb_p[:])
            et_t = {1: cp.tile([128, NCH], F32, tag="et1", name="et1_t"),
                    2: cp.tile([128, NCH], F32, tag="et2", name="et2_t")}
            for L in (1, 2):
                nc.vector.tensor_scalar_mul(et_t[L][:], ea_t[:], csb[:, L - 1:L])

            out_sb = cp.tile([128, NBLK * 129], F32)
            h_all = cp.tile([128, NBLK * 128], F32)
            aq_all = cp.tile([128, 3 * NBLK], F32)
            bias_bc = cp.tile([128, 128], F32)

            qrr = [0]

            def qn():
                qrr[0] = (qrr[0] + 1) % 4
                return qrr[0]

            for L in (1, 2):
                # ---- bias broadcast [128,128] ----
                pb = px.tile([128, 128], F32, tag="pmisc")
                nc.tensor.matmul(pb[:], lhsT=ones1[:], rhs=B_t[L][:], start=True, stop=True)
                nc.vector.tensor_copy(bias_bc[:], pb[:])

                # ---- node transform table build ----
                for t in range(NBLK):
                    if L == 1:
                        lhs = xT_t[:, t * 128:(t + 1) * 128]
                    else:
                        pT = px.tile([128, 128], F32, tag="pmisc")
                        nc.tensor.transpose(pT[:], h_all[:, t * 128:(t + 1) * 128], ident[:])
                        hT = wp.tile([128, 128], F16, tag="hT")
                        nc.vector.tensor_copy(hT[:], pT[:])
                        lhs = hT[:]
                    ptab = pt.tile([128, 393], F32)
                    nc.tensor.matmul(ptab[:], lhsT=lhs, rhs=W_t[L][:], start=True, stop=True)
                    stab = wp.tile([128, 390], BF16, tag="stab")
                    nc.vector.tensor_copy(stab[:], ptab[:, 0:390])
                    for r in range(3):
                        nc.vector.memset(stab[:, r * 130 + 128:r * 130 + 129], 1.0)
                        nc.vector.tensor_copy(aq_all[:, r * NBLK + t:r * NBLK + t + 1],
                                              ptab[:, 390 + r:391 + r])
                    for r in range(3):
                        nc.sync.dma_start(
                            out=tabs[L][r * NPC + t * 128:r * NPC + (t + 1) * 128, 0:130],
                            in_=stab[:, r * 130:r * 130 + 130])
                for r in range(3):
                    dstv = aqt[L][r * NPC:(r + 1) * NPC, 0:1] \
                        .rearrange("(t p) o -> p (t o)", p=128)
                    nc.sync.dma_start(out=dstv, in_=aq_all[:, r * NBLK:(r + 1) * NBLK])

                # ---- AllGather the table ----
                nc.gpsimd.collective_compute(
                    "AllGather", AL.bypass, replica_groups=[list(range(cfg.NC))],
                    ins=[tabs[L][:]], outs=[tabg[L][:]])

                # ---- main edge loop ----
                nc.vector.memset(out_sb[:], 0.0)
                call_tiles = {}
                expa_tiles = {}
                for (p, s0, ns) in cfg.calls:
                    vrows = min(cfg.RANGE, cfg.RTOT - p * cfg.RANGE)
                    fst = sp.tile([128, cfg.GCALL, 130], BF16, tag="fst")
                    if 'gather' in skips:
                        nc.vector.memset(fst[:, 0, 0:2], 0.0)
                    else: nc.gpsimd.dma_gather(
                        fst[:, :ns, :],
                        tabg[L][p * cfg.RANGE:p * cfg.RANGE + vrows, 0:130],
                        fidx_t[:, s0 * 8:(s0 + ns) * 8],
                        ns * 128, ns * 128, 130, elem_step=256,
                        single_packet=False, queue_num=qn())
                    aqs = qp.tile([128, cfg.GCALL, 1], F32, tag="aqs")
                    if 'aq' in skips:
                        nc.vector.memset(aqs[:, 0, 0:1], 0.0)
                    else: nc.gpsimd.dma_gather(
                        aqs[:, :ns, :], aqt[L][:, 0:1],
                        aqix_t[:, s0 * 8:(s0 + ns) * 8],
                        ns * 128, ns * 128, 1, elem_step=64,
                        single_packet=False, queue_num=qn())
                    ext = qp.tile([128, cfg.GCALL], F32, tag="ext")
                    sl = ext[:, :ns]
                    if 'alpha' in skips:
                        nc.vector.memset(ext[:, 0:2], 0.0)

                    if 'alpha' not in skips:
                        akf = wp.tile([128, cfg.GCALL], F32, tag="akf")
                        nc.vector.tensor_copy(akf[:, :ns], fst[:, :ns, 129])
                        nc.vector.tensor_tensor(sl, aqs[:, :ns, 0], akf[:, :ns], op=AL.add)
                        nc.vector.tensor_tensor(sl, sl, et_t[L][:, s0:s0 + ns], op=AL.add)
                        lrt = wp.tile([128, cfg.GCALL], F32, tag="lrt")
                        nc.vector.tensor_scalar_mul(lrt[:, :ns], sl, NEG_SLOPE)
                        nc.vector.tensor_tensor(sl, sl, lrt[:, :ns], op=AL.max)
                        nc.scalar.activation(sl, sl, AF.Exp)
                    for k in range(ns):
                        call_tiles[s0 + k] = (fst, k)
                        expa_tiles[s0 + k] = (ext, k)

                for grp in [(p,) for p in range(cfg.NPH)]:
                    for b in range(NBLK):
                        slots = [int(cfg.base[p] + cfg.pboff[p, b] + c)
                                 for p in grp for c in range(int(cfg.CPB[b, p]))]
                        if not slots:
                            continue
                        pacc = pa.tile([128, 129], F32)
                        if 'mm' in skips:
                            nc.vector.memset(pacc[:, 0:2], 0.0)
                        for ci, s in enumerate(slots):
                            fst, ls = call_tiles[s]
                            oa = op.tile([128, 128], BF16, tag="oa")
                            ext, ek = expa_tiles[s]
                            if 'oa' in skips:
                                nc.vector.memset(oa[:, 0:2], 0.0)
                            if 'oa' not in skips:
                                nc.vector.tensor_scalar(
                                    oa[:], iof[:], dst_t[:, s:s + 1], ext[:, ek:ek + 1],
                                    op0=AL.is_equal, op1=AL.mult)
                            if 'mm' not in skips:
                                nc.tensor.matmul(pacc[:], lhsT=oa[:], rhs=fst[:, ls, 0:129],
                                                 start=(ci == 0), stop=(ci == len(slots) - 1))
                        if 'evac' not in skips:
                            nc.vector.tensor_tensor(out_sb[:, b * 129:(b + 1) * 129],
                                                    out_sb[:, b * 129:(b + 1) * 129],
                                                    pacc[:], op=AL.add)

                # ---- finalize ----
                for b in range(NBLK):
                    rc = wp.tile([128, 1], F32, tag="rc")
                    nc.vector.tensor_scalar_add(rc[:], out_sb[:, b * 129 + 128:b * 129 + 129],
                                                1e-16)
                    nc.vector.reciprocal(rc[:], rc[:])
                    if L == 1:
                        tgt = h_all[:, b * 128:(b + 1) * 128]
                    else:
                        ot = wp.tile([128, 128], F32, tag="ot")
                        tgt = ot[:]
                    nc.vector.tensor_scalar_mul(tgt, out_sb[:, b * 129:b * 129 + 128], rc[:])
                    nc.vector.tensor_tensor(tgt, tgt, bias_bc[:], op=AL.add)
                    if L == 1:
                        nc.vector.tensor_scalar_max(tgt, tgt, 0.0)
                    else:
                        o16 = wp.tile([128, 128], F16, tag="o16")
                        nc.vector.tensor_copy(o16[:], tgt)
                        nc.sync.dma_start(out=OUT2[b * 128:(b + 1) * 128, :], in_=o16[:])
    nc.compile()
    return nc


# ---------------------------------------------------------------------------
# Cached PJRT runner: jit once, keep NEFF zero-output buffers device-resident.
# ---------------------------------------------------------------------------
_CACHE = {}


def _build_runner(nc, n_cores):
    import jax
    import jax.numpy as jnp
    from jax.sharding import Mesh, PartitionSpec, NamedSharding
    from jax.experimental.shard_map import shard_map

    bass2jax.install_neuronx_cc_hook()
    partition_name = nc.partition_id_tensor.name if nc.partition_id_tensor else None
    in_names, out_names, out_avals = [], [], []
    for alloc in nc.m.functions[0].allocations:
        if not isinstance(alloc, mybir.MemoryLocationSet):
            continue
        name = alloc.memorylocations[0].name
        if alloc.kind == "ExternalInput":
            if name != partition_name:
                in_names.append(name)
        elif alloc.kind == "ExternalOutput":
            out_names.append(name)
            out_avals.append(jax.core.ShapedArray(tuple(alloc.tensor_shape),
                                                  mybir.dt.np(alloc.dtype)))
    n_params = len(in_names)
    in_names_all = in_names + out_names + ([partition_name] if partition_name else [])

    def _body(*args):
        operands = list(args)
        if partition_name is not None:
            operands.append(bass2jax.partition_id_tensor())
        outs = bass2jax._bass_exec_p.bind(
            *operands, out_avals=tuple(out_avals), in_names=tuple(in_names_all),
            out_names=tuple(out_names), lowering_input_output_aliases=(),
            sim_require_finite=True, sim_require_nnan=True, nc=nc)
        return tuple(outs)

    devices = jax.devices()[:n_cores]
    assert len(devices) == n_cores
    mesh = Mesh(np.asarray(devices), ("core",))
    nspec = NamedSharding(mesh, PartitionSpec("core"))
    in_specs = (PartitionSpec("core"),) * (n_params + len(out_names))
    out_specs = (PartitionSpec("core"),) * len(out_names)
    sharded = jax.jit(shard_map(_body, mesh=mesh, in_specs=in_specs,
                                out_specs=out_specs, check_rep=False),
                      keep_unused=True)
    # The kernel writes every element of every output, so the "zero" NEFF
    # output buffers are never observed: keep one device-resident copy.
    dev_zeros = [jax.device_put(
        np.zeros((n_cores * av.shape[0], *av.shape[1:]), av.dtype), nspec)
        for av in out_avals]

    def _gather_input(per_core, n):
        parts = [per_core[c][n] for c in range(n_cores)]
        b = parts[0].base
        if b is not None and all(p.base is b for p in parts) and b.flags['C_CONTIGUOUS']:
            r = parts[0].shape[0]
            if (b.shape == (n_cores * r,) + parts[0].shape[1:]
                    and b.dtype == parts[0].dtype):
                a0 = b.__array_interface__['data'][0]
                if all(p.__array_interface__['data'][0] == a0 + c * p.nbytes
                       for c, p in enumerate(parts)):
                    return b
        return np.concatenate(parts, axis=0)

    def run(per_core):
        concat_in = [_gather_input(per_core, n) for n in in_names]
        outs = sharded(*concat_in, *dev_zeros)
        return [np.asarray(o).reshape(n_cores, *out_avals[i].shape)
                for i, o in enumerate(outs)]

    return run


def get_runner(cfg):
    key = (cfg.N, cfg.E, cfg.NCH, int(cfg.CPB.sum()),
           tuple(int(x) for x in cfg.base))
    if key not in _CACHE:
        nc = build_nc(cfg)
        _CACHE[key] = _build_runner(nc, cfg.NC)
    return _CACHE[key]


def run(x, edge_index, edge_type, edge_attr, w1, q1, k1, le1, e1, b1,
        w2, q2, k2, le2, e2, b2, N=None, E=None):
    x = np.asarray(x, np.float32)
    N = x.shape[0] if N is None else N
    E = edge_index.shape[1] if E is None else E
    cfg = make_cfg(N, E)
    per_core = host_prep(cfg, x, np.asarray(edge_index), np.asarray(edge_type),
                         np.asarray(edge_attr, np.float32),
                         np.asarray(w1, np.float32), np.asarray(q1, np.float32),
                         np.asarray(k1, np.float32), np.asarray(le1, np.float32),
                         np.asarray(e1, np.float32), np.asarray(b1, np.float32),
                         np.asarray(w2, np.float32), np.asarray(q2, np.float32),
                         np.asarray(k2, np.float32), np.asarray(le2, np.float32),
                         np.asarray(e2, np.float32), np.asarray(b2, np.float32))
    runner = get_runner(cfg)
    out = runner(per_core)[0].reshape(-1, 128)
    return out[:N]


def kernel(**inputs):
    return run(
        inputs["x"], inputs["edge_index"], inputs["edge_type"], inputs["edge_attr"],
        inputs["w1"], inputs["q1"], inputs["k1"], inputs["le1"], inputs["e1"], inputs["b1"],
        inputs["w2"], inputs["q2"], inputs["k2"], inputs["le2"], inputs["e2"], inputs["b2"],
    ).astype(np.float32)


# revision 12
# speedup vs baseline: 1.0154x; 1.0154x over previous
"""Two-layer RGAT (R=3, heads=1) on 8 trn2 NeuronCores.

Strategy (dst-sharded, one-hot-matmul aggregation):
  - Nodes padded to 50176 = 8 cores x 49 blocks x 128; core c owns dst nodes
    [c*6272, (c+1)*6272) and computes the full output rows for them.
  - Per layer, each core computes its slice of the per-relation node transform
    xw[r] = x @ W_r (plus attention scalars ak = xw@k, aq = xw@q) into a DRAM
    table (row = (src_core, rt, src_local), 192-f32 stride, 130 payload:
    [128 feats | 1.0 | ak]); AllGather replicates the table.
  - Edges (sorted by dst block, then by table-row range so int16 gather
    indices fit) are processed in 128-edge chunks: dma_gather fetches the
    chunk's source rows; alpha = exp(LeakyRelu(aq[rt,dst] + ak[rt,src] +
    c_l*ea)) is built from a second (local) aq-table gather; a fused DVE
    tensor_scalar builds the alpha-scaled one-hot O[e, dst_local]; one
    matmul per chunk accumulates psum[node,129] = [sum alpha*xj | sum alpha].
  - Block results accumulate in SBUF across range-phases; finalize divides by
    the denominator, adds bias (+ReLU for layer 1). Layer-2 output rows DMA
    straight to the per-core output; the host concatenates and trims.

I/O path (the axon tunnel moves ~60-90 MB/s, so host<->device bytes dominate):
  - Inputs are sent compact: xT/W packs in fp16, dst-locals as int16, raw
    edge_attr as fp16 (scaled by c1/c2 on device), gather-index tiles as a
    single 16-partition copy (the 8x SWDGE replication is done on-device).
  - The output is fp16.
  - The jitted executable is cached; the NEFF's zero output buffers live on
    device persistently (no donation - the kernel writes every output elem).
"""
import sys
sys.path.insert(0, '/opt/trn_rl_repo')
import inspect
import textwrap
import numpy as np

import concourse.bass as bass
import concourse.bacc as bacc
import concourse.mybir as mybir
from concourse import bass2jax
from concourse.tile import TileContext
from concourse.masks import make_identity

F32 = mybir.dt.float32
F16 = mybir.dt.float16
I16 = mybir.dt.int16
I32 = mybir.dt.int32
NEG_SLOPE = 0.2

# ---- relax dma_gather's elem_size%256 restriction (descriptor length is ----
# ---- arbitrary; only the row *stride* must be a multiple of 256B)       ----
_src = inspect.getsource(bass.BassGpSimd.dma_gather)
_src = _src.replace(
    "elem_size_bytes > 0 and elem_size_bytes % 256 == 0",
    "elem_size_bytes > 0",
)
_ns = {}
exec(compile(textwrap.dedent(_src), "<dma_gather_patched>", "exec"), dict(vars(bass)), _ns)
bass.BassGpSimd.dma_gather = _ns["dma_gather"]


class Cfg:
    pass


def make_cfg(N, E, NC=8, GCALL=32, RANGE=32768):
    cfg = Cfg()
    cfg.NC = NC
    cfg.N, cfg.E = N, E
    cfg.NPAD = -(-N // (128 * NC)) * 128 * NC
    cfg.NPC = cfg.NPAD // NC
    cfg.NBLK = cfg.NPC // 128
    cfg.RPC = 3 * cfg.NPC
    cfg.RTOT = cfg.RPC * NC
    cfg.RANGE = RANGE
    cfg.NPH = -(-cfg.RTOT // RANGE)
    cfg.GCALL = GCALL
    return cfg


def host_prep(cfg, x, edge_index, edge_type, edge_attr, w1, q1, k1, le1, e1, b1,
              w2, q2, k2, le2, e2, b2):
    """Returns (per_core_inputs list, cfg with CPB/calls/NCH set)."""
    NC, NPC, NBLK, RANGE = cfg.NC, cfg.NPC, cfg.NBLK, cfg.RANGE
    src, dst = edge_index[0].astype(np.int64), edge_index[1].astype(np.int64)
    rt = edge_type.astype(np.int64)
    ea = edge_attr[:, 0].astype(np.float32)
    c1 = float(le1.reshape(-1) @ e1.reshape(-1))
    c2 = float(le2.reshape(-1) @ e2.reshape(-1))

    core = dst // NPC
    blk = (dst % NPC) // 128
    dl = dst % 128
    grow = (src // NPC) * cfg.RPC + rt * NPC + (src % NPC)
    ph = grow // RANGE
    lidx = grow - ph * RANGE
    aqi = rt * NPC + (dst % NPC)

    # per (core, blk, phase) counts -> CPB[p][b] = max-over-cores chunks
    counts = np.zeros((NC, NBLK, cfg.NPH), np.int64)
    np.add.at(counts, (core, blk, ph), 1)
    CPB = -(-counts.max(axis=0) // 128)          # [NBLK, NPH]
    cfg.CPB = CPB
    # slot layout: phase-major; within phase, blocks at cumsum offsets
    cfg.pboff = np.zeros((cfg.NPH, NBLK), np.int64)
    base = [0]
    for p in range(cfg.NPH):
        cfg.pboff[p] = np.concatenate([[0], np.cumsum(CPB[:-1, p])])
        base.append(base[-1] + int(CPB[:, p].sum()))
    cfg.base = np.asarray(base, np.int64)
    cfg.NCH = int(cfg.base[-1])

    # gather call list: per phase, contiguous slot runs of <= GCALL slots
    calls = []
    for p in range(cfg.NPH):
        nslots = int(CPB[:, p].sum())
        s = 0
        while s < nslots:
            ns = min(cfg.GCALL, nslots - s)
            calls.append((p, int(cfg.base[p] + s), int(ns)))
            s += ns
    cfg.calls = calls

    def pack16(vals):
        """vals [NCH*128] -> compact idx tile [16, NCH*8] (one SWDGE copy;
        the device replicates to the 8 required partition groups)."""
        out = np.zeros((16, cfg.NCH * 8), np.int16)
        for (p, s0, ns) in calls:
            v = vals[s0 * 128:(s0 + ns) * 128]
            i = np.arange(ns * 128)
            out[i % 16, s0 * 8 + i // 16] = v
        return out

    # weight packs (fp16)
    def wpack(w, qv, kv):
        W = np.zeros((128, 393), np.float32)
        for r in range(3):
            W[:, r * 130:r * 130 + 128] = w[r]
            W[:, r * 130 + 129] = (w[r] @ kv).ravel()
            W[:, 390 + r] = (w[r] @ qv).ravel()
        return W.astype(np.float16)

    W1p, W2p = wpack(w1, q1, k1), wpack(w2, q2, k2)

    # Concat-shaped base arrays; per-core dicts hold row-slice views so the
    # runner can skip the per-run np.concatenate memcpy.
    full = {
        "xT": np.zeros((NC * 128, NPC), np.float16),
        "W1": np.zeros((NC * 128, 393), np.float16),
        "W2": np.zeros((NC * 128, 393), np.float16),
        "B1": np.zeros((NC * 1, 128), np.float32),
        "B2": np.zeros((NC * 1, 128), np.float32),
        "CSC": np.zeros((NC * 1, 2), np.float32),
        "DSTS": np.full((NC * 128, cfg.NCH), -1, np.int16),
        "EAS": np.zeros((NC * 128, cfg.NCH), np.float16),
        "FIDX": np.zeros((NC * 16, cfg.NCH * 8), np.int16),
        "AQIX": np.zeros((NC * 16, cfg.NCH * 8), np.int16),
    }

    per_core = []
    for c in range(NC):
        m = core == c
        eb, ep = blk[m], ph[m]
        edl, elx, eaq = dl[m], lidx[m], aqi[m]
        eea = ea[m]
        order = np.lexsort((ep, eb))
        eb, ep, edl, elx, eaq, eea = (a[order] for a in (eb, ep, edl, elx, eaq, eea))
        # rank within (blk, phase) group
        gid = eb * cfg.NPH + ep
        boundaries = np.concatenate([[0], np.cumsum(np.bincount(gid.astype(np.int64),
                                                                minlength=NBLK * cfg.NPH))])
        rank = np.arange(len(gid)) - boundaries[gid]
        slot = cfg.base[ep] + cfg.pboff[ep, eb] + rank // 128
        prow = rank % 128

        dst_s = full["DSTS"][c * 128:(c + 1) * 128]
        ea_s = full["EAS"][c * 128:(c + 1) * 128]
        fidx_v = np.zeros(cfg.NCH * 128, np.int64)
        aq_v = np.zeros(cfg.NCH * 128, np.int64)
        dst_s[prow, slot] = edl
        ea_s[prow, slot] = eea
        fidx_v[slot * 128 + prow] = elx
        aq_v[slot * 128 + prow] = eaq
        full["FIDX"][c * 16:(c + 1) * 16] = pack16(fidx_v)
        full["AQIX"][c * 16:(c + 1) * 16] = pack16(aq_v)

        lo, hi = c * NPC, min((c + 1) * NPC, cfg.N)
        if hi > lo:
            full["xT"][c * 128:(c + 1) * 128, :hi - lo] = x[lo:hi].T.astype(np.float16)
        full["W1"][c * 128:(c + 1) * 128] = W1p
        full["W2"][c * 128:(c + 1) * 128] = W2p
        full["B1"][c] = b1.reshape(128).astype(np.float32)
        full["B2"][c] = b2.reshape(128).astype(np.float32)
        full["CSC"][c] = (c1, c2)
        per_core.append({k: (full[k][c * 16:(c + 1) * 16] if k in ("FIDX", "AQIX")
                             else full[k][c:c + 1] if k in ("B1", "B2", "CSC")
                             else full[k][c * 128:(c + 1) * 128])
                         for k in full})
    return per_core


def build_nc(cfg, skips=()):
    skips = set(skips)
    nc = bacc.Bacc("TRN2", target_bir_lowering=False, num_swdge_queues=4)
    NPC, NBLK, NCH = cfg.NPC, cfg.NBLK, cfg.NCH

    xT = nc.declare_dram_parameter("xT", [128, NPC], F16, isOutput=False)
    W = {1: nc.declare_dram_parameter("W1", [128, 393], F16, isOutput=False),
         2: nc.declare_dram_parameter("W2", [128, 393], F16, isOutput=False)}
    B = {1: nc.declare_dram_parameter("B1", [1, 128], F32, isOutput=False),
         2: nc.declare_dram_parameter("B2", [1, 128], F32, isOutput=False)}
    CSC = nc.declare_dram_parameter("CSC", [1, 2], F32, isOutput=False)
    DSTS = nc.declare_dram_parameter("DSTS", [128, NCH], I16, isOutput=False)
    EAS = nc.declare_dram_parameter("EAS", [128, NCH], F16, isOutput=False)
    FIDX = nc.declare_dram_parameter("FIDX", [16, NCH * 8], I16, isOutput=False)
    AQIX = nc.declare_dram_parameter("AQIX", [16, NCH * 8], I16, isOutput=False)
    OUT2 = nc.declare_dram_parameter("out2", [NPC, 128], F16, isOutput=True)

    tabs = {L: nc.dram_tensor(f"tabs{L}", [cfg.RPC, 192], F32) for L in (1, 2)}
    tabg = {L: nc.dram_tensor(f"tabg{L}", [cfg.RTOT, 192], F32, addr_space="Shared")
            for L in (1, 2)}
    aqt = {L: nc.dram_tensor(f"aqt{L}", [cfg.RPC, 64], F32) for L in (1, 2)}

    AL = mybir.AluOpType
    AF = mybir.ActivationFunctionType

    with TileContext(nc) as tc:
        with (
            tc.tile_pool(name="const", bufs=1) as cp,
            tc.tile_pool(name="stag", bufs=4) as sp,
            tc.tile_pool(name="aqs", bufs=6) as qp,
            tc.tile_pool(name="oa", bufs=8) as op,
            tc.tile_pool(name="work", bufs=3) as wp,
            tc.tile_pool(name="pacc", bufs=4, space="PSUM") as pa,
            tc.tile_pool(name="ptab", bufs=2, space="PSUM") as pt,
            tc.tile_pool(name="pmisc", bufs=2, space="PSUM") as px,
        ):
            # ---- constants / staged inputs ----
            xT_t = cp.tile([128, NPC], F16)
            nc.sync.dma_start(out=xT_t[:], in_=xT[:])
            W_t = {L: cp.tile([128, 393], F16, tag=f"W{L}", name=f"W{L}_t") for L in (1, 2)}
            B_t = {L: cp.tile([1, 128], F32, tag=f"B{L}", name=f"B{L}_t") for L in (1, 2)}
            for L in (1, 2):
                nc.sync.dma_start(out=W_t[L][:], in_=W[L][:])
                nc.sync.dma_start(out=B_t[L][:], in_=B[L][:])
            csc_t = cp.tile([1, 2], F32)
            nc.sync.dma_start(out=csc_t[:], in_=CSC[:])
            dsti_t = cp.tile([128, NCH], I16)
            nc.sync.dma_start(out=dsti_t[:], in_=DSTS[:])
            ea_t = cp.tile([128, NCH], F16)
            nc.sync.dma_start(out=ea_t[:], in_=EAS[:])
            fidx_t = cp.tile([128, NCH * 8], I16)
            aqix_t = cp.tile([128, NCH * 8], I16)
            for g in range(8):
                nc.sync.dma_start(out=fidx_t[16 * g:16 * (g + 1), :], in_=FIDX[:])
                nc.sync.dma_start(out=aqix_t[16 * g:16 * (g + 1), :], in_=AQIX[:])

            ii = cp.tile([128, 128], I32)
            nc.gpsimd.iota(ii[:], pattern=[[1, 128]], base=0, channel_multiplier=0)
            iof = cp.tile([128, 128], F32)
            nc.vector.tensor_copy(iof[:], ii[:])
            ident = cp.tile([128, 128], F32)
            make_identity(nc, ident[:])
            ones1 = cp.tile([1, 128], F32)
            nc.vector.memset(ones1[:], 1.0)

            # dst-locals as f32; per-layer scaled edge attrs
            dst_t = cp.tile([128, NCH], F32)
            nc.vector.tensor_copy(dst_t[:], dsti_t[:])
            csb_p = px.tile([128, 2], F32, tag="pmisc")
            nc.tensor.matmul(csb_p[:], lhsT=ones1[:], rhs=csc_t[:], start=True, stop=True)
            csb = cp.tile([128, 2], F32)
            nc.vector.tensor_copy(csb[:], csb_p[:])
            et_t = {1: cp.tile([128, NCH], F32, tag="et1", name="et1_t"),
                    2: cp.tile([128, NCH], F32, tag="et2", name="et2_t")}
            for L in (1, 2):
                nc.vector.tensor_scalar_mul(et_t[L][:], ea_t[:], csb[:, L - 1:L])

            out_sb = cp.tile([128, NBLK * 129], F32)
            h_all = cp.tile([128, NBLK * 128], F32)
            aq_all = cp.tile([128, 3 * NBLK], F32)
            bias_bc = cp.tile([128, 128], F32)

            qrr = [0]

            def qn():
                qrr[0] = (qrr[0] + 1) % 4
                return qrr[0]

            for L in (1, 2):
                # ---- bias broadcast [128,128] ----
                pb = px.tile([128, 128], F32, tag="pmisc")
                nc.tensor.matmul(pb[:], lhsT=ones1[:], rhs=B_t[L][:], start=True, stop=True)
                nc.vector.tensor_copy(bias_bc[:], pb[:])

                # ---- node transform table build ----
                for t in range(NBLK):
                    if L == 1:
                        lhs = xT_t[:, t * 128:(t + 1) * 128]
                    else:
                        pT = px.tile([128, 128], F32, tag="pmisc")
                        nc.tensor.transpose(pT[:], h_all[:, t * 128:(t + 1) * 128], ident[:])
                        hT = wp.tile([128, 128], F16, tag="hT")
                        nc.vector.tensor_copy(hT[:], pT[:])
                        lhs = hT[:]
                    ptab = pt.tile([128, 393], F32)
                    nc.tensor.matmul(ptab[:], lhsT=lhs, rhs=W_t[L][:], start=True, stop=True)
                    stab = wp.tile([128, 390], F32, tag="stab")
                    nc.vector.tensor_copy(stab[:], ptab[:, 0:390])
                    for r in range(3):
                        nc.vector.memset(stab[:, r * 130 + 128:r * 130 + 129], 1.0)
                        nc.vector.tensor_copy(aq_all[:, r * NBLK + t:r * NBLK + t + 1],
                                              ptab[:, 390 + r:391 + r])
                    for r in range(3):
                        nc.sync.dma_start(
                            out=tabs[L][r * NPC + t * 128:r * NPC + (t + 1) * 128, 0:130],
                            in_=stab[:, r * 130:r * 130 + 130])
                for r in range(3):
                    dstv = aqt[L][r * NPC:(r + 1) * NPC, 0:1] \
                        .rearrange("(t p) o -> p (t o)", p=128)
                    nc.sync.dma_start(out=dstv, in_=aq_all[:, r * NBLK:(r + 1) * NBLK])

                # ---- AllGather the table ----
                nc.gpsimd.collective_compute(
                    "AllGather", AL.bypass, replica_groups=[list(range(cfg.NC))],
                    ins=[tabs[L][:]], outs=[tabg[L][:]])

                # ---- main edge loop ----
                nc.vector.memset(out_sb[:], 0.0)
                call_tiles = {}
                expa_tiles = {}
                for (p, s0, ns) in cfg.calls:
                    vrows = min(cfg.RANGE, cfg.RTOT - p * cfg.RANGE)
                    fst = sp.tile([128, cfg.GCALL, 130], F32, tag="fst")
                    if 'gather' in skips:
                        nc.vector.memset(fst[:, 0, 0:2], 0.0)
                    else: nc.gpsimd.dma_gather(
                        fst[:, :ns, :],
                        tabg[L][p * cfg.RANGE:p * cfg.RANGE + vrows, 0:130],
                        fidx_t[:, s0 * 8:(s0 + ns) * 8],
                        ns * 128, ns * 128, 130, elem_step=192,
                        single_packet=False, queue_num=qn())
                    aqs = qp.tile([128, cfg.GCALL, 1], F32, tag="aqs")
                    if 'aq' in skips:
                        nc.vector.memset(aqs[:, 0, 0:1], 0.0)
                    else: nc.gpsimd.dma_gather(
                        aqs[:, :ns, :], aqt[L][:, 0:1],
                        aqix_t[:, s0 * 8:(s0 + ns) * 8],
                        ns * 128, ns * 128, 1, elem_step=64,
                        single_packet=False, queue_num=qn())
                    ext = qp.tile([128, cfg.GCALL], F32, tag="ext")
                    sl = ext[:, :ns]
                    if 'alpha' in skips:
                        nc.vector.memset(ext[:, 0:2], 0.0)
                    if 'alpha' not in skips:
                        nc.vector.tensor_tensor(sl, aqs[:, :ns, 0], fst[:, :ns, 129], op=AL.add)
                        nc.vector.tensor_tensor(sl, sl, et_t[L][:, s0:s0 + ns], op=AL.add)
                        lrt = wp.tile([128, cfg.GCALL], F32, tag="lrt")
                        nc.vector.tensor_scalar_mul(lrt[:, :ns], sl, NEG_SLOPE)
                        nc.vector.tensor_tensor(sl, sl, lrt[:, :ns], op=AL.max)
                        nc.scalar.activation(sl, sl, AF.Exp)
                    for k in range(ns):
                        call_tiles[s0 + k] = (fst, k)
                        expa_tiles[s0 + k] = (ext, k)

                for grp in [(p,) for p in range(cfg.NPH)]:
                    for b in range(NBLK):
                        slots = [int(cfg.base[p] + cfg.pboff[p, b] + c)
                                 for p in grp for c in range(int(cfg.CPB[b, p]))]
                        if not slots:
                            continue
                        pacc = pa.tile([128, 129], F32)
                        if 'mm' in skips:
                            nc.vector.memset(pacc[:, 0:2], 0.0)
                        for ci, s in enumerate(slots):
                            fst, ls = call_tiles[s]
                            oa = op.tile([128, 128], F32, tag="oa")
                            ext, ek = expa_tiles[s]
                            if 'oa' in skips:
                                nc.vector.memset(oa[:, 0:2], 0.0)
                            if 'oa' not in skips:
                                nc.vector.tensor_scalar(
                                    oa[:], iof[:], dst_t[:, s:s + 1], ext[:, ek:ek + 1],
                                    op0=AL.is_equal, op1=AL.mult)
                            if 'mm' not in skips:
                                nc.tensor.matmul(pacc[:], lhsT=oa[:], rhs=fst[:, ls, 0:129],
                                                 start=(ci == 0), stop=(ci == len(slots) - 1))
                        if 'evac' not in skips:
                            nc.vector.tensor_tensor(out_sb[:, b * 129:(b + 1) * 129],
                                                    out_sb[:, b * 129:(b + 1) * 129],
                                                    pacc[:], op=AL.add)

                # ---- finalize ----
                for b in range(NBLK):
                    rc = wp.tile([128, 1], F32, tag="rc")
                    nc.vector.tensor_scalar_add(rc[:], out_sb[:, b * 129 + 128:b * 129 + 129],
                                                1e-16)
                    nc.vector.reciprocal(rc[:], rc[:])
                    if L == 1:
                        tgt = h_all[:, b * 128:(b + 1) * 128]
                    else:
                        ot = wp.tile([128, 128], F32, tag="ot")
                        tgt = ot[:]
                    nc.vector.tensor_scalar_mul(tgt, out_sb[:, b * 129:b * 129 + 128], rc[:])
                    nc.vector.tensor_tensor(tgt, tgt, bias_bc[:], op=AL.add)
                    if L == 1:
                        nc.vector.tensor_scalar_max(tgt, tgt, 0.0)
                    else:
                        o16 = wp.tile([128, 128], F16, tag="o16")
                        nc.vector.tensor_copy(o16[:], tgt)
                        nc.sync.dma_start(out=OUT2[b * 128:(b + 1) * 128, :], in_=o16[:])
    nc.compile()
    return nc


# ---------------------------------------------------------------------------
# Cached PJRT runner: jit once, keep NEFF zero-output buffers device-resident.
# ---------------------------------------------------------------------------
_CACHE = {}


def _build_runner(nc, n_cores):
    import jax
    import jax.numpy as jnp
    from jax.sharding import Mesh, PartitionSpec, NamedSharding
    from jax.experimental.shard_map import shard_map

    bass2jax.install_neuronx_cc_hook()
    partition_name = nc.partition_id_tensor.name if nc.partition_id_tensor else None
    in_names, out_names, out_avals = [], [], []
    for alloc in nc.m.functions[0].allocations:
        if not isinstance(alloc, mybir.MemoryLocationSet):
            continue
        name = alloc.memorylocations[0].name
        if alloc.kind == "ExternalInput":
            if name != partition_name:
                in_names.append(name)
        elif alloc.kind == "ExternalOutput":
            out_names.append(name)
            out_avals.append(jax.core.ShapedArray(tuple(alloc.tensor_shape),
                                                  mybir.dt.np(alloc.dtype)))
    n_params = len(in_names)
    in_names_all = in_names + out_names + ([partition_name] if partition_name else [])

    def _body(*args):
        operands = list(args)
        if partition_name is not None:
            operands.append(bass2jax.partition_id_tensor())
        outs = bass2jax._bass_exec_p.bind(
            *operands, out_avals=tuple(out_avals), in_names=tuple(in_names_all),
            out_names=tuple(out_names), lowering_input_output_aliases=(),
            sim_require_finite=True, sim_require_nnan=True, nc=nc)
        return tuple(outs)

    devices = jax.devices()[:n_cores]
    assert len(devices) == n_cores
    mesh = Mesh(np.asarray(devices), ("core",))
    nspec = NamedSharding(mesh, PartitionSpec("core"))
    in_specs = (PartitionSpec("core"),) * (n_params + len(out_names))
    out_specs = (PartitionSpec("core"),) * len(out_names)
    sharded = jax.jit(shard_map(_body, mesh=mesh, in_specs=in_specs,
                                out_specs=out_specs, check_rep=False),
                      keep_unused=True)
    # The kernel writes every element of every output, so the "zero" NEFF
    # output buffers are never observed: keep one device-resident copy.
    dev_zeros = [jax.device_put(
        np.zeros((n_cores * av.shape[0], *av.shape[1:]), av.dtype), nspec)
        for av in out_avals]

    def _gather_input(per_core, n):
        parts = [per_core[c][n] for c in range(n_cores)]
        b = parts[0].base
        if b is not None and all(p.base is b for p in parts) and b.flags['C_CONTIGUOUS']:
            r = parts[0].shape[0]
            if (b.shape == (n_cores * r,) + parts[0].shape[1:]
                    and b.dtype == parts[0].dtype):
                a0 = b.__array_interface__['data'][0]
                if all(p.__array_interface__['data'][0] == a0 + c * p.nbytes
                       for c, p in enumerate(parts)):
                    return b
        return np.concatenate(parts, axis=0)

    def run(per_core):
        concat_in = [_gather_input(per_core, n) for n in in_names]
        outs = sharded(*concat_in, *dev_zeros)
        return [np.asarray(o).reshape(n_cores, *out_avals[i].shape)
                for i, o in enumerate(outs)]

    return run


def get_runner(cfg):
    key = (cfg.N, cfg.E, cfg.NCH, int(cfg.CPB.sum()),
           tuple(int(x) for x in cfg.base))
    if key not in _CACHE:
        nc = build_nc(cfg)
        _CACHE[key] = _build_runner(nc, cfg.NC)
    return _CACHE[key]


def run(x, edge_index, edge_type, edge_attr, w1, q1, k1, le1, e1, b1,
        w2, q2, k2, le2, e2, b2, N=None, E=None):
    x = np.asarray(x, np.float32)
    N = x.shape[0] if N is None else N
    E = edge_index.shape[1] if E is None else E
    cfg = make_cfg(N, E)
    per_core = host_prep(cfg, x, np.asarray(edge_index), np.asarray(edge_type),
                         np.asarray(edge_attr, np.float32),
                         np.asarray(w1, np.float32), np.asarray(q1, np.float32),
                         np.asarray(k1, np.float32), np.asarray(le1, np.float32),
                         np.asarray(e1, np.float32), np.asarray(b1, np.float32),
                         np.asarray(w2, np.float32), np.asarray(q2, np.float32),
                         np.asarray(k2, np.float32), np.asarray(le2, np.float32),
                         np.asarray(e2, np.float32), np.asarray(b2, np.float32))
    runner = get_runner(cfg)
    out = runner(per_core)[0].reshape(-1, 128)
    return out[:N]


def kernel(**inputs):
    return run(
        inputs["x"], inputs["edge_index"], inputs["edge_type"], inputs["edge_attr"],
        inputs["w1"], inputs["q1"], inputs["k1"], inputs["le1"], inputs["e1"], inputs["b1"],
        inputs["w2"], inputs["q2"], inputs["k2"], inputs["le2"], inputs["e2"], inputs["b2"],
    ).astype(np.float32)


# revision 19
# speedup vs baseline: 1.0511x; 1.0352x over previous
"""Two-layer RGAT (R=3, heads=1) on 8 trn2 NeuronCores.

Strategy (dst-sharded, one-hot-matmul aggregation):
  - Nodes padded to 50176 = 8 cores x 49 blocks x 128; core c owns dst nodes
    [c*6272, (c+1)*6272) and computes the full output rows for them.
  - Per layer, each core computes its slice of the per-relation node transform
    xw[r] = x @ W_r (plus attention scalars ak = xw@k, aq = xw@q) into a DRAM
    table (row = (src_core, rt, src_local), 192-f32 stride, 130 payload:
    [128 feats | 1.0 | ak]); AllGather replicates the table.
  - Edges (sorted by dst block, then by table-row range so int16 gather
    indices fit) are processed in 128-edge chunks: dma_gather fetches the
    chunk's source rows; alpha = exp(LeakyRelu(aq[rt,dst] + ak[rt,src] +
    c_l*ea)) is built from a second (local) aq-table gather; a fused DVE
    tensor_scalar builds the alpha-scaled one-hot O[e, dst_local]; one
    matmul per chunk accumulates psum[node,129] = [sum alpha*xj | sum alpha].
  - Block results accumulate in SBUF across range-phases; finalize divides by
    the denominator, adds bias (+ReLU for layer 1). Layer-2 output rows DMA
    straight to the per-core output; the host concatenates and trims.

I/O path (the axon tunnel moves ~60-90 MB/s, so host<->device bytes dominate):
  - Inputs are sent compact: xT/W packs in fp16, dst-locals as int16, raw
    edge_attr as fp16 (scaled by c1/c2 on device), gather-index tiles as a
    single 16-partition copy (the 8x SWDGE replication is done on-device).
  - The output is fp16.
  - The jitted executable is cached; the NEFF's zero output buffers live on
    device persistently (no donation - the kernel writes every output elem).
"""
import sys
sys.path.insert(0, '/opt/trn_rl_repo')
import inspect
import textwrap
import numpy as np

import concourse.bass as bass
import concourse.bacc as bacc
import concourse.mybir as mybir
from concourse import bass2jax
from concourse.tile import TileContext
from concourse.masks import make_identity

F32 = mybir.dt.float32
F16 = mybir.dt.float16
I16 = mybir.dt.int16
I32 = mybir.dt.int32
NEG_SLOPE = 0.2

# ---- relax dma_gather's elem_size%256 restriction (descriptor length is ----
# ---- arbitrary; only the row *stride* must be a multiple of 256B)       ----
_src = inspect.getsource(bass.BassGpSimd.dma_gather)
_src = _src.replace(
    "elem_size_bytes > 0 and elem_size_bytes % 256 == 0",
    "elem_size_bytes > 0",
)
_ns = {}
exec(compile(textwrap.dedent(_src), "<dma_gather_patched>", "exec"), dict(vars(bass)), _ns)
bass.BassGpSimd.dma_gather = _ns["dma_gather"]


class Cfg:
    pass


def make_cfg(N, E, NC=8, GCALL=32, RANGE=32768):
    cfg = Cfg()
    cfg.NC = NC
    cfg.N, cfg.E = N, E
    cfg.NPAD = -(-N // (128 * NC)) * 128 * NC
    cfg.NPC = cfg.NPAD // NC
    cfg.NBLK = cfg.NPC // 128
    cfg.RPC = 3 * cfg.NPC
    cfg.RTOT = cfg.RPC * NC
    cfg.RANGE = RANGE
    cfg.NPH = -(-cfg.RTOT // RANGE)
    cfg.GCALL = GCALL
    return cfg


def host_prep(cfg, x, edge_index, edge_type, edge_attr, w1, q1, k1, le1, e1, b1,
              w2, q2, k2, le2, e2, b2):
    """Returns (per_core_inputs list, cfg with CPB/calls/NCH set)."""
    NC, NPC, NBLK, RANGE = cfg.NC, cfg.NPC, cfg.NBLK, cfg.RANGE
    src, dst = edge_index[0].astype(np.int64), edge_index[1].astype(np.int64)
    rt = edge_type.astype(np.int64)
    ea = edge_attr[:, 0].astype(np.float32)
    c1 = float(le1.reshape(-1) @ e1.reshape(-1))
    c2 = float(le2.reshape(-1) @ e2.reshape(-1))

    core = dst // NPC
    blk = (dst % NPC) // 128
    dl = dst % 128
    grow = (src // NPC) * cfg.RPC + rt * NPC + (src % NPC)
    ph = grow // RANGE
    lidx = grow - ph * RANGE
    aqi = rt * NPC + (dst % NPC)

    # per (core, blk, phase) counts -> CPB[p][b] = max-over-cores chunks
    counts = np.zeros((NC, NBLK, cfg.NPH), np.int64)
    np.add.at(counts, (core, blk, ph), 1)
    CPB = -(-counts.max(axis=0) // 128)          # [NBLK, NPH]
    cfg.CPB = CPB
    # slot layout: phase-major; within phase, blocks at cumsum offsets
    cfg.pboff = np.zeros((cfg.NPH, NBLK), np.int64)
    base = [0]
    for p in range(cfg.NPH):
        cfg.pboff[p] = np.concatenate([[0], np.cumsum(CPB[:-1, p])])
        base.append(base[-1] + int(CPB[:, p].sum()))
    cfg.base = np.asarray(base, np.int64)
    cfg.NCH = int(cfg.base[-1])

    # gather call list: per phase, contiguous slot runs of <= GCALL slots
    calls = []
    for p in range(cfg.NPH):
        nslots = int(CPB[:, p].sum())
        s = 0
        while s < nslots:
            ns = min(cfg.GCALL, nslots - s)
            calls.append((p, int(cfg.base[p] + s), int(ns)))
            s += ns
    cfg.calls = calls

    def pack16(vals):
        """vals [NCH*128] -> compact idx tile [16, NCH*8] (one SWDGE copy;
        the device replicates to the 8 required partition groups)."""
        out = np.zeros((16, cfg.NCH * 8), np.int16)
        for (p, s0, ns) in calls:
            v = vals[s0 * 128:(s0 + ns) * 128]
            i = np.arange(ns * 128)
            out[i % 16, s0 * 8 + i // 16] = v
        return out

    # weight packs (fp16)
    def wpack(w, qv, kv):
        W = np.zeros((128, 393), np.float32)
        for r in range(3):
            W[:, r * 130:r * 130 + 128] = w[r]
            W[:, r * 130 + 129] = (w[r] @ kv).ravel()
            W[:, 390 + r] = (w[r] @ qv).ravel()
        return W.astype(np.float16)

    W1p, W2p = wpack(w1, q1, k1), wpack(w2, q2, k2)

    # Concat-shaped base arrays; per-core dicts hold row-slice views so the
    # runner can skip the per-run np.concatenate memcpy.
    full = {
        "xT0": np.zeros((NC * 128, NPC // 4), np.float16),
        "xT1": np.zeros((NC * 128, NPC // 4), np.float16),
        "xT2": np.zeros((NC * 128, NPC // 4), np.float16),
        "xT3": np.zeros((NC * 128, NPC // 4), np.float16),
        "W1": np.zeros((NC * 128, 393), np.float16),
        "W2": np.zeros((NC * 128, 393), np.float16),
        "B1": np.zeros((NC * 1, 128), np.float32),
        "B2": np.zeros((NC * 1, 128), np.float32),
        "CSC": np.zeros((NC * 1, 2), np.float32),
        "DSTS": np.full((NC * 128, cfg.NCH), -1, np.int8),
        "EAS": np.zeros((NC * 128, cfg.NCH), np.float16),
        "FIDX": np.zeros((NC * 16, cfg.NCH * 8), np.int16),
        "AQIX": np.zeros((NC * 16, cfg.NCH * 8), np.int16),
    }

    per_core = []
    for c in range(NC):
        m = core == c
        eb, ep = blk[m], ph[m]
        edl, elx, eaq = dl[m], lidx[m], aqi[m]
        eea = ea[m]
        order = np.lexsort((ep, eb))
        eb, ep, edl, elx, eaq, eea = (a[order] for a in (eb, ep, edl, elx, eaq, eea))
        # rank within (blk, phase) group
        gid = eb * cfg.NPH + ep
        boundaries = np.concatenate([[0], np.cumsum(np.bincount(gid.astype(np.int64),
                                                                minlength=NBLK * cfg.NPH))])
        rank = np.arange(len(gid)) - boundaries[gid]
        slot = cfg.base[ep] + cfg.pboff[ep, eb] + rank // 128
        prow = rank % 128

        dst_s = full["DSTS"][c * 128:(c + 1) * 128]
        ea_s = full["EAS"][c * 128:(c + 1) * 128]
        fidx_v = np.zeros(cfg.NCH * 128, np.int64)
        aq_v = np.zeros(cfg.NCH * 128, np.int64)
        dst_s[prow, slot] = edl
        ea_s[prow, slot] = eea
        fidx_v[slot * 128 + prow] = elx
        aq_v[slot * 128 + prow] = eaq
        full["FIDX"][c * 16:(c + 1) * 16] = pack16(fidx_v)
        full["AQIX"][c * 16:(c + 1) * 16] = pack16(aq_v)

        lo, hi = c * NPC, min((c + 1) * NPC, cfg.N)
        if hi > lo:
            xtc = np.zeros((128, NPC), np.float16)
            xtc[:, :hi - lo] = x[lo:hi].T.astype(np.float16)
            for q in range(4):
                full[f"xT{q}"][c * 128:(c + 1) * 128] = \
                    xtc[:, q * (NPC // 4):(q + 1) * (NPC // 4)]
        full["W1"][c * 128:(c + 1) * 128] = W1p
        full["W2"][c * 128:(c + 1) * 128] = W2p
        full["B1"][c] = b1.reshape(128).astype(np.float32)
        full["B2"][c] = b2.reshape(128).astype(np.float32)
        full["CSC"][c] = (c1, c2)
        per_core.append({k: (full[k][c * 16:(c + 1) * 16] if k in ("FIDX", "AQIX")
                             else full[k][c:c + 1] if k in ("B1", "B2", "CSC")
                             else full[k][c * 128:(c + 1) * 128])
                         for k in full})
    return per_core


def build_nc(cfg, skips=()):
    skips = set(skips)
    nc = bacc.Bacc("TRN2", target_bir_lowering=False, num_swdge_queues=4)
    NPC, NBLK, NCH = cfg.NPC, cfg.NBLK, cfg.NCH

    xTq = [nc.declare_dram_parameter(f"xT{q}", [128, NPC // 4], F16, isOutput=False)
           for q in range(4)]
    W = {1: nc.declare_dram_parameter("W1", [128, 393], F16, isOutput=False),
         2: nc.declare_dram_parameter("W2", [128, 393], F16, isOutput=False)}
    B = {1: nc.declare_dram_parameter("B1", [1, 128], F32, isOutput=False),
         2: nc.declare_dram_parameter("B2", [1, 128], F32, isOutput=False)}
    CSC = nc.declare_dram_parameter("CSC", [1, 2], F32, isOutput=False)
    DSTS = nc.declare_dram_parameter("DSTS", [128, NCH], mybir.dt.int8, isOutput=False)
    EAS = nc.declare_dram_parameter("EAS", [128, NCH], F16, isOutput=False)
    FIDX = nc.declare_dram_parameter("FIDX", [16, NCH * 8], I16, isOutput=False)
    AQIX = nc.declare_dram_parameter("AQIX", [16, NCH * 8], I16, isOutput=False)
    OUT2 = nc.declare_dram_parameter("out2", [NPC, 128], F16, isOutput=True)

    tabs = {L: nc.dram_tensor(f"tabs{L}", [cfg.RPC, 192], F32) for L in (1, 2)}
    tabg = {L: nc.dram_tensor(f"tabg{L}", [cfg.RTOT, 192], F32, addr_space="Shared")
            for L in (1, 2)}
    aqt = {L: nc.dram_tensor(f"aqt{L}", [cfg.RPC, 64], F32) for L in (1, 2)}

    AL = mybir.AluOpType
    AF = mybir.ActivationFunctionType

    with TileContext(nc) as tc:
        with (
            tc.tile_pool(name="const", bufs=1) as cp,
            tc.tile_pool(name="stag", bufs=4) as sp,
            tc.tile_pool(name="aqs", bufs=6) as qp,
            tc.tile_pool(name="oa", bufs=8) as op,
            tc.tile_pool(name="work", bufs=3) as wp,
            tc.tile_pool(name="pacc", bufs=4, space="PSUM") as pa,
            tc.tile_pool(name="ptab", bufs=2, space="PSUM") as pt,
            tc.tile_pool(name="pmisc", bufs=2, space="PSUM") as px,
        ):
            # ---- constants / staged inputs ----
            xT_t = cp.tile([128, NPC], F16)
            for q in range(4):
                nc.sync.dma_start(out=xT_t[:, q * (NPC // 4):(q + 1) * (NPC // 4)],
                                  in_=xTq[q][:])
            W_t = {L: cp.tile([128, 393], F16, tag=f"W{L}", name=f"W{L}_t") for L in (1, 2)}
            B_t = {L: cp.tile([1, 128], F32, tag=f"B{L}", name=f"B{L}_t") for L in (1, 2)}
            for L in (1, 2):
                nc.sync.dma_start(out=W_t[L][:], in_=W[L][:])
                nc.sync.dma_start(out=B_t[L][:], in_=B[L][:])
            csc_t = cp.tile([1, 2], F32)
            nc.sync.dma_start(out=csc_t[:], in_=CSC[:])
            dsti_t = cp.tile([128, NCH], mybir.dt.int8)
            nc.sync.dma_start(out=dsti_t[:], in_=DSTS[:])
            ea_t = cp.tile([128, NCH], F16)
            nc.sync.dma_start(out=ea_t[:], in_=EAS[:])
            fidx_t = cp.tile([128, NCH * 8], I16)
            aqix_t = cp.tile([128, NCH * 8], I16)
            for g in range(8):
                nc.sync.dma_start(out=fidx_t[16 * g:16 * (g + 1), :], in_=FIDX[:])
                nc.sync.dma_start(out=aqix_t[16 * g:16 * (g + 1), :], in_=AQIX[:])

            ii = cp.tile([128, 128], I32)
            nc.gpsimd.iota(ii[:], pattern=[[1, 128]], base=0, channel_multiplier=0)
            iof = cp.tile([128, 128], F32)
            nc.vector.tensor_copy(iof[:], ii[:])
            ident = cp.tile([128, 128], F32)
            make_identity(nc, ident[:])
            ones1 = cp.tile([1, 128], F32)
            nc.vector.memset(ones1[:], 1.0)

            # dst-locals as f32; per-layer scaled edge attrs
            dst_t = cp.tile([128, NCH], F32)
            nc.vector.tensor_copy(dst_t[:], dsti_t[:])
            csb_p = px.tile([128, 2], F32, tag="pmisc")
            nc.tensor.matmul(csb_p[:], lhsT=ones1[:], rhs=csc_t[:], start=True, stop=True)
            csb = cp.tile([128, 2], F32)
            nc.vector.tensor_copy(csb[:], csb_p[:])
            et_t = {1: cp.tile([128, NCH], F32, tag="et1", name="et1_t"),
                    2: cp.tile([128, NCH], F32, tag="et2", name="et2_t")}
            for L in (1, 2):
                nc.vector.tensor_scalar_mul(et_t[L][:], ea_t[:], csb[:, L - 1:L])

            out_sb = cp.tile([128, NBLK * 129], F32)
            h_all = cp.tile([128, NBLK * 128], F32)
            aq_all = cp.tile([128, 3 * NBLK], F32)
            bias_bc = cp.tile([128, 128], F32)

            qrr = [0]

            def qn():
                qrr[0] = (qrr[0] + 1) % 4
                return qrr[0]

            for L in (1, 2):
                # ---- bias broadcast [128,128] ----
                pb = px.tile([128, 128], F32, tag="pmisc")
                nc.tensor.matmul(pb[:], lhsT=ones1[:], rhs=B_t[L][:], start=True, stop=True)
                nc.vector.tensor_copy(bias_bc[:], pb[:])

                # ---- node transform table build ----
                for t in range(NBLK):
                    if L == 1:
                        lhs = xT_t[:, t * 128:(t + 1) * 128]
                    else:
                        pT = px.tile([128, 128], F32, tag="pmisc")
                        nc.tensor.transpose(pT[:], h_all[:, t * 128:(t + 1) * 128], ident[:])
                        hT = wp.tile([128, 128], F16, tag="hT")
                        nc.vector.tensor_copy(hT[:], pT[:])
                        lhs = hT[:]
                    ptab = pt.tile([128, 393], F32)
                    nc.tensor.matmul(ptab[:], lhsT=lhs, rhs=W_t[L][:], start=True, stop=True)
                    stab = wp.tile([128, 390], F32, tag="stab")
                    nc.vector.tensor_copy(stab[:], ptab[:, 0:390])
                    for r in range(3):
                        nc.vector.memset(stab[:, r * 130 + 128:r * 130 + 129], 1.0)
                        nc.vector.tensor_copy(aq_all[:, r * NBLK + t:r * NBLK + t + 1],
                                              ptab[:, 390 + r:391 + r])
                    for r in range(3):
                        nc.sync.dma_start(
                            out=tabs[L][r * NPC + t * 128:r * NPC + (t + 1) * 128, 0:130],
                            in_=stab[:, r * 130:r * 130 + 130])
                for r in range(3):
                    dstv = aqt[L][r * NPC:(r + 1) * NPC, 0:1] \
                        .rearrange("(t p) o -> p (t o)", p=128)
                    nc.sync.dma_start(out=dstv, in_=aq_all[:, r * NBLK:(r + 1) * NBLK])

                # ---- AllGather the table ----
                nc.gpsimd.collective_compute(
                    "AllGather", AL.bypass, replica_groups=[list(range(cfg.NC))],
                    ins=[tabs[L][:]], outs=[tabg[L][:]])

                # ---- main edge loop ----
                nc.vector.memset(out_sb[:], 0.0)
                call_tiles = {}
                expa_tiles = {}
                for (p, s0, ns) in cfg.calls:
                    vrows = min(cfg.RANGE, cfg.RTOT - p * cfg.RANGE)
                    fst = sp.tile([128, cfg.GCALL, 130], F32, tag="fst")
                    if 'gather' in skips:
                        nc.vector.memset(fst[:, 0, 0:2], 0.0)
                    else: nc.gpsimd.dma_gather(
                        fst[:, :ns, :],
                        tabg[L][p * cfg.RANGE:p * cfg.RANGE + vrows, 0:130],
                        fidx_t[:, s0 * 8:(s0 + ns) * 8],
                        ns * 128, ns * 128, 130, elem_step=192,
                        single_packet=False, queue_num=qn())
                    aqs = qp.tile([128, cfg.GCALL, 1], F32, tag="aqs")
                    if 'aq' in skips:
                        nc.vector.memset(aqs[:, 0, 0:1], 0.0)
                    else: nc.gpsimd.dma_gather(
                        aqs[:, :ns, :], aqt[L][:, 0:1],
                        aqix_t[:, s0 * 8:(s0 + ns) * 8],
                        ns * 128, ns * 128, 1, elem_step=64,
                        single_packet=False, queue_num=qn())
                    ext = qp.tile([128, cfg.GCALL], F32, tag="ext")
                    sl = ext[:, :ns]
                    if 'alpha' in skips:
                        nc.vector.memset(ext[:, 0:2], 0.0)
                    if 'alpha' not in skips:
                        nc.vector.tensor_tensor(sl, aqs[:, :ns, 0], fst[:, :ns, 129], op=AL.add)
                        nc.vector.tensor_tensor(sl, sl, et_t[L][:, s0:s0 + ns], op=AL.add)
                        lrt = wp.tile([128, cfg.GCALL], F32, tag="lrt")
                        nc.vector.tensor_scalar_mul(lrt[:, :ns], sl, NEG_SLOPE)
                        nc.vector.tensor_tensor(sl, sl, lrt[:, :ns], op=AL.max)
                        nc.scalar.activation(sl, sl, AF.Exp)
                    for k in range(ns):
                        call_tiles[s0 + k] = (fst, k)
                        expa_tiles[s0 + k] = (ext, k)

                for grp in [(p,) for p in range(cfg.NPH)]:
                    for b in range(NBLK):
                        slots = [int(cfg.base[p] + cfg.pboff[p, b] + c)
                                 for p in grp for c in range(int(cfg.CPB[b, p]))]
                        if not slots:
                            continue
                        pacc = pa.tile([128, 129], F32)
                        if 'mm' in skips:
                            nc.vector.memset(pacc[:, 0:2], 0.0)
                        for ci, s in enumerate(slots):
                            fst, ls = call_tiles[s]
                            oa = op.tile([128, 128], F32, tag="oa")
                            ext, ek = expa_tiles[s]
                            if 'oa' in skips:
                                nc.vector.memset(oa[:, 0:2], 0.0)
                            if 'oa' not in skips:
                                nc.vector.tensor_scalar(
                                    oa[:], iof[:], dst_t[:, s:s + 1], ext[:, ek:ek + 1],
                                    op0=AL.is_equal, op1=AL.mult)
                            if 'mm' not in skips:
                                nc.tensor.matmul(pacc[:], lhsT=oa[:], rhs=fst[:, ls, 0:129],
                                                 start=(ci == 0), stop=(ci == len(slots) - 1))
                        if 'evac' not in skips:
                            nc.vector.tensor_tensor(out_sb[:, b * 129:(b + 1) * 129],
                                                    out_sb[:, b * 129:(b + 1) * 129],
                                                    pacc[:], op=AL.add)

                # ---- finalize ----
                for b in range(NBLK):
                    rc = wp.tile([128, 1], F32, tag="rc")
                    nc.vector.tensor_scalar_add(rc[:], out_sb[:, b * 129 + 128:b * 129 + 129],
                                                1e-16)
                    nc.vector.reciprocal(rc[:], rc[:])
                    if L == 1:
                        tgt = h_all[:, b * 128:(b + 1) * 128]
                    else:
                        ot = wp.tile([128, 128], F32, tag="ot")
                        tgt = ot[:]
                    nc.vector.tensor_scalar_mul(tgt, out_sb[:, b * 129:b * 129 + 128], rc[:])
                    nc.vector.tensor_tensor(tgt, tgt, bias_bc[:], op=AL.add)
                    if L == 1:
                        nc.vector.tensor_scalar_max(tgt, tgt, 0.0)
                    else:
                        o16 = wp.tile([128, 128], F16, tag="o16")
                        nc.vector.tensor_copy(o16[:], tgt)
                        nc.sync.dma_start(out=OUT2[b * 128:(b + 1) * 128, :], in_=o16[:])
    nc.compile()
    return nc


# ---------------------------------------------------------------------------
# Cached PJRT runner: jit once, keep NEFF zero-output buffers device-resident.
# ---------------------------------------------------------------------------
_CACHE = {}


def _build_runner(nc, n_cores):
    import jax
    import jax.numpy as jnp
    from jax.sharding import Mesh, PartitionSpec, NamedSharding
    from jax.experimental.shard_map import shard_map

    bass2jax.install_neuronx_cc_hook()
    partition_name = nc.partition_id_tensor.name if nc.partition_id_tensor else None
    in_names, out_names, out_avals = [], [], []
    for alloc in nc.m.functions[0].allocations:
        if not isinstance(alloc, mybir.MemoryLocationSet):
            continue
        name = alloc.memorylocations[0].name
        if alloc.kind == "ExternalInput":
            if name != partition_name:
                in_names.append(name)
        elif alloc.kind == "ExternalOutput":
            out_names.append(name)
            out_avals.append(jax.core.ShapedArray(tuple(alloc.tensor_shape),
                                                  mybir.dt.np(alloc.dtype)))
    n_params = len(in_names)
    in_names_all = in_names + out_names + ([partition_name] if partition_name else [])

    def _body(*args):
        operands = list(args)
        if partition_name is not None:
            operands.append(bass2jax.partition_id_tensor())
        outs = bass2jax._bass_exec_p.bind(
            *operands, out_avals=tuple(out_avals), in_names=tuple(in_names_all),
            out_names=tuple(out_names), lowering_input_output_aliases=(),
            sim_require_finite=True, sim_require_nnan=True, nc=nc)
        return tuple(outs)

    devices = jax.devices()[:n_cores]
    assert len(devices) == n_cores
    mesh = Mesh(np.asarray(devices), ("core",))
    nspec = NamedSharding(mesh, PartitionSpec("core"))
    in_specs = (PartitionSpec("core"),) * (n_params + len(out_names))
    out_specs = (PartitionSpec("core"),) * len(out_names)
    sharded = jax.jit(shard_map(_body, mesh=mesh, in_specs=in_specs,
                                out_specs=out_specs, check_rep=False),
                      keep_unused=True)
    # The kernel writes every element of every output, so the "zero" NEFF
    # output buffers are never observed: keep one device-resident copy.
    dev_zeros = [jax.device_put(
        np.zeros((n_cores * av.shape[0], *av.shape[1:]), av.dtype), nspec)
        for av in out_avals]

    def _gather_input(per_core, n):
        parts = [per_core[c][n] for c in range(n_cores)]
        b = parts[0].base
        if b is not None and all(p.base is b for p in parts) and b.flags['C_CONTIGUOUS']:
            r = parts[0].shape[0]
            if (b.shape == (n_cores * r,) + parts[0].shape[1:]
                    and b.dtype == parts[0].dtype):
                a0 = b.__array_interface__['data'][0]
                if all(p.__array_interface__['data'][0] == a0 + c * p.nbytes
                       for c, p in enumerate(parts)):
                    return b
        return np.concatenate(parts, axis=0)

    def run(per_core):
        concat_in = [_gather_input(per_core, n) for n in in_names]
        outs = sharded(*concat_in, *dev_zeros)
        return [np.asarray(o).reshape(n_cores, *out_avals[i].shape)
                for i, o in enumerate(outs)]

    return run


def get_runner(cfg):
    key = (cfg.N, cfg.E, cfg.NCH, int(cfg.CPB.sum()),
           tuple(int(x) for x in cfg.base))
    if key not in _CACHE:
        nc = build_nc(cfg)
        _CACHE[key] = _build_runner(nc, cfg.NC)
    return _CACHE[key]


def run(x, edge_index, edge_type, edge_attr, w1, q1, k1, le1, e1, b1,
        w2, q2, k2, le2, e2, b2, N=None, E=None):
    x = np.asarray(x, np.float32)
    N = x.shape[0] if N is None else N
    E = edge_index.shape[1] if E is None else E
    cfg = make_cfg(N, E)
    per_core = host_prep(cfg, x, np.asarray(edge_index), np.asarray(edge_type),
                         np.asarray(edge_attr, np.float32),
                         np.asarray(w1, np.float32), np.asarray(q1, np.float32),
                         np.asarray(k1, np.float32), np.asarray(le1, np.float32),
                         np.asarray(e1, np.float32), np.asarray(b1, np.float32),
                         np.asarray(w2, np.float32), np.asarray(q2, np.float32),
                         np.asarray(k2, np.float32), np.asarray(le2, np.float32),
                         np.asarray(e2, np.float32), np.asarray(b2, np.float32))
    runner = get_runner(cfg)
    out = runner(per_core)[0].reshape(-1, 128)
    return out[:N]


def kernel(**inputs):
    return run(
        inputs["x"], inputs["edge_index"], inputs["edge_type"], inputs["edge_attr"],
        inputs["w1"], inputs["q1"], inputs["k1"], inputs["le1"], inputs["e1"], inputs["b1"],
        inputs["w2"], inputs["q2"], inputs["k2"], inputs["le2"], inputs["e2"], inputs["b2"],
    ).astype(np.float32)
